# revision 8
# baseline (speedup 1.0000x reference)
"""Bayer mosaic channel selection on 8 Trainium2 NeuronCores.

Reference computes out[b, i, j] = img[b, c(i,j), i, j] with
    c = 1 where (i+j) even
    c = 2 where i even and j odd
    c = 0 where i odd and j even

So each output row interleaves two channels at element granularity:
    even rows:  ch1 @ even cols, ch2 @ odd cols
    odd rows:   ch0 @ even cols, ch1 @ odd cols

Sharding: pure data-parallel, one batch image per NeuronCore (B == 8).

Per-core plan (rows processed in blocks of 256 = 128 even + 128 odd):
  - Two strided-row DMA loads per block pull only the rows that are used
    (ch1 all rows, ch2 even rows, ch0 odd rows -> 2/3 of the input), with
    row parities de-interleaved so each SBUF partition holds one output row.
  - Four stride-2 vector-engine copies assemble the output rows in a
    separate SBUF area (each output element is touched exactly once).
  - One DMA store re-interleaves row parities back to HBM.
Every DMA moves 2 MiB in 8 KiB contiguous chunks, so transfers stay near
line rate; the kernel is HBM-bandwidth-bound as intended.

Written in raw Bass (not Tile): walrus codegen caps the number of packed
sync-wait conditions per instruction at two, which Tile's auto-generated
semaphores exceed for this dependency pattern. Raw streams emit waits as
standalone instructions, sidestepping the cap, and triple buffering gives
load/compute/store overlap.
"""

from contextlib import ExitStack

import numpy as np

import concourse.bass as bass
import concourse.mybir as mybir

B, C, H, W = 8, 3, 2048, 2048
P = 128           # SBUF partitions = output row pairs per block
RB = 2 * P        # image rows per block
NBLK = H // RB
NBUF = 3          # triple buffering: load / compute / store in flight

_NC_CACHE: list = []


def build_nc():
    f32 = mybir.dt.float32
    nc = bass.Bass()
    img = nc.declare_dram_parameter("img", [C, H, W], f32, isOutput=False)
    out = nc.declare_dram_parameter("out", [H, W], f32, isOutput=True)

    # [row-pair, parity, channel, col] and [row-pair, parity, col] views
    img_r = img.rearrange("c (p two) w -> p two c w", two=2)
    out_r = out.rearrange("(p two) w -> p two w", two=2)

    with ExitStack() as ctx:
        ctx.enter_context(nc.cleanup_on_exit())
        # te: seg0 = ch1 @ even rows, seg1 = ch2 @ even rows
        # to: seg0 = ch0 @ odd rows,  seg1 = ch1 @ odd rows
        # ob: seg0 = even output rows, seg1 = odd output rows
        te = [
            ctx.enter_context(nc.sbuf_tensor(f"te{i}", [P, 2 * W], f32))
            for i in range(NBUF)
        ]
        to = [
            ctx.enter_context(nc.sbuf_tensor(f"to{i}", [P, 2 * W], f32))
            for i in range(NBUF)
        ]
        ob = [
            ctx.enter_context(nc.sbuf_tensor(f"ob{i}", [P, 2 * W], f32))
            for i in range(NBUF)
        ]
        # Per-buffer-slot DMA semaphores: a shared counting sem would be
        # ambiguous with several DMAs in flight (sub-completion increments
        # from different transfers interleave), which CoreSim's race
        # detector rightly rejects. sem_cp is single-producer in-order DVE.
        sem_in = [
            ctx.enter_context(nc.semaphore(f"sem_in{i}")) for i in range(NBUF)
        ]
        sem_st = [
            ctx.enter_context(nc.semaphore(f"sem_st{i}")) for i in range(NBUF)
        ]
        sem_cp = ctx.enter_context(nc.semaphore("sem_cp"))

        with nc.Block() as block:

            @block.sync
            def _(sync):
                for k in range(NBLK):
                    j, r = k % NBUF, k // NBUF
                    if k >= NBUF:
                        # input buffers free once block k-NBUF's copies ran
                        sync.wait_ge(sem_cp, 4 * (k - NBUF) + 4)
                        # acknowledge prior increments before re-incrementing
                        sync.wait_ge(sem_in[j], 32 * r)
                    pr0 = k * P
                    sync.dma_start(
                        out=te[j].rearrange("p (c w) -> p c w", w=W),
                        in_=img_r[pr0 : pr0 + P, 0, 1:3, :],
                    ).then_inc(sem_in[j], 16)
                    sync.dma_start(
                        out=to[j].rearrange("p (c w) -> p c w", w=W),
                        in_=img_r[pr0 : pr0 + P, 1, 0:2, :],
                    ).then_inc(sem_in[j], 16)

            @block.vector
            def _(vector):
                for k in range(NBLK):
                    j, r = k % NBUF, k // NBUF
                    vector.wait_ge(sem_in[j], 32 * (r + 1))
                    if k >= NBUF:
                        # output buffer free once block k-NBUF's store ran
                        vector.wait_ge(sem_st[j], 16 * r)
                    t_e, t_o, o = te[j], to[j], ob[j]
                    # even rows: ch1 @ even cols, ch2 @ odd cols
                    vector.tensor_copy(o[:, 0:W:2], t_e[:, 0:W:2]).then_inc(sem_cp, 1)
                    vector.tensor_copy(
                        o[:, 1:W:2], t_e[:, W + 1 : 2 * W : 2]
                    ).then_inc(sem_cp, 1)
                    # odd rows: ch0 @ even cols, ch1 @ odd cols
                    vector.tensor_copy(
                        o[:, W : 2 * W : 2], t_o[:, 0:W:2]
                    ).then_inc(sem_cp, 1)
                    vector.tensor_copy(
                        o[:, W + 1 : 2 * W : 2], t_o[:, W + 1 : 2 * W : 2]
                    ).then_inc(sem_cp, 1)

            @block.scalar
            def _(scalar):
                for k in range(NBLK):
                    j, r = k % NBUF, k // NBUF
                    scalar.wait_ge(sem_cp, 4 * (k + 1))
                    if k >= NBUF:
                        scalar.wait_ge(sem_st[j], 16 * r)
                    pr0 = k * P
                    scalar.dma_start(
                        out=out_r[pr0 : pr0 + P],
                        in_=ob[j].rearrange("p (s w) -> p s w", s=2),
                    ).then_inc(sem_st[j], 16)

    return nc


def _get_nc():
    if not _NC_CACHE:
        _NC_CACHE.append(build_nc())
    return _NC_CACHE[0]


def kernel(**inputs) -> np.ndarray:
    img = np.asarray(inputs["img"], dtype=np.float32)
    assert img.shape == (B, C, H, W), img.shape

    from concourse.bass_utils import run_bass_kernel_spmd

    nc = _get_nc()
    in_maps = [{"img": np.ascontiguousarray(img[b])} for b in range(B)]
    res = run_bass_kernel_spmd(nc, in_maps, core_ids=list(range(B)))
    return np.stack([res.results[i]["out"] for i in range(B)], axis=0)


# revision 21
# speedup vs baseline: 1.1633x; 1.1633x over previous
"""Bayer mosaic channel selection on 8 Trainium2 NeuronCores.

Reference computes out[b, i, j] = img[b, c(i,j), i, j] with
    c = 1 where (i+j) even
    c = 2 where i even and j odd
    c = 0 where i odd and j even

So each output row interleaves two channels at element granularity:
    even rows:  ch1 @ even cols, ch2 @ odd cols
    odd rows:   ch0 @ even cols, ch1 @ odd cols

Sharding: pure data-parallel, one batch image per NeuronCore (B == 8).

Per-core plan (rows processed in blocks of 256 = 128 even + 128 odd):
  - Two strided-row 2 MiB DMA loads per block pull only the rows that are
    used (ch1 all rows, ch2 even rows, ch0 odd rows -> 2/3 of the input),
    with row parities de-interleaved so each SBUF partition holds one
    output row.
  - Four stride-2 vector-engine copies assemble the output rows in a
    separate SBUF area (each output element is touched exactly once).
  - One 2 MiB DMA store re-interleaves row parities back to HBM.
All DMA traffic moves in 8 KiB contiguous chunks, so transfers stay near
line rate; the kernel is HBM-bandwidth-bound as intended.

Written in raw Bass (not Tile): walrus codegen caps the number of packed
sync-wait conditions per instruction at two, which Tile's auto-generated
semaphores exceed for this dependency pattern. Raw streams emit waits as
standalone instructions, sidestepping the cap, and triple buffering gives
load/compute/store overlap.
"""

from contextlib import ExitStack

import numpy as np

import concourse.bass as bass
import concourse.mybir as mybir

B, C, H, W = 8, 3, 2048, 2048
P = 128           # SBUF partitions = output row pairs per block
RB = 2 * P        # image rows per block
NBLK = H // RB
NBUF = 3          # triple buffering: load / compute / store in flight

_NC_CACHE: list = []


def build_nc():
    f32 = mybir.dt.float32
    nc = bass.Bass()
    img = nc.declare_dram_parameter("img", [C, H, W], f32, isOutput=False)
    out = nc.declare_dram_parameter("out", [H, W], f32, isOutput=True)

    # [row-pair, parity, col] view of the output
    out_r = out.rearrange("(p two) w -> p two w", two=2)

    # The four used row-streams of a block, as element offsets relative to
    # r0*W (r0 = first image row of the block), stepping 2W per partition:
    #     ch0 @ odd rows   ->  W
    #     ch1 @ even rows  ->  W + (HW - W)
    #     ch1 @ odd rows   ->  W + HW
    #     ch2 @ even rows  ->  W + HW + (HW - W)
    # Two rectangular [step HW-W, count 2] pairs -> two 2 MiB DMAs per
    # block (the DMA AP balancer caps access patterns at 3 dims, so the
    # four streams cannot merge into a single 4 MiB transfer).

    with ExitStack() as ctx:
        ctx.enter_context(nc.cleanup_on_exit())
        # tin: seg0 = ch0 @ odd rows, seg1 = ch1 @ even rows,
        #      seg2 = ch1 @ odd rows, seg3 = ch2 @ even rows
        # ob:  seg0 = even output rows, seg1 = odd output rows
        tin = [
            ctx.enter_context(nc.sbuf_tensor(f"tin{i}", [P, 4 * W], f32))
            for i in range(NBUF)
        ]
        ob = [
            ctx.enter_context(nc.sbuf_tensor(f"ob{i}", [P, 2 * W], f32))
            for i in range(NBUF)
        ]
        # Per-buffer-slot DMA semaphores: a shared counting sem would be
        # ambiguous with several DMAs in flight (sub-completion increments
        # from different transfers interleave), which CoreSim's race
        # detector rightly rejects. sem_cp is single-producer in-order DVE.
        sem_in = [
            ctx.enter_context(nc.semaphore(f"sem_in{i}")) for i in range(NBUF)
        ]
        sem_st = [
            ctx.enter_context(nc.semaphore(f"sem_st{i}")) for i in range(NBUF)
        ]
        sem_cp = ctx.enter_context(nc.semaphore("sem_cp"))

        with nc.Block() as block:

            @block.sync
            def _(sync):
                for k in range(NBLK):
                    j, r = k % NBUF, k // NBUF
                    if k >= NBUF:
                        # input buffer free once block k-NBUF's copies ran
                        sync.wait_ge(sem_cp, 4 * (k - NBUF) + 4)
                        # acknowledge prior increments before re-incrementing
                        sync.wait_ge(sem_in[j], 32 * r)
                    base = k * RB * W + W
                    src_a = bass.AP(
                        img, base, [[2 * W, P], [H * W - W, 2], [1, W]]
                    )
                    src_b = bass.AP(
                        img, base + H * W, [[2 * W, P], [H * W - W, 2], [1, W]]
                    )
                    sync.dma_start(
                        out=tin[j][:, 0 : 2 * W].rearrange("p (s w) -> p s w", w=W),
                        in_=src_a,
                    ).then_inc(sem_in[j], 16)
                    sync.dma_start(
                        out=tin[j][:, 2 * W : 4 * W].rearrange("p (s w) -> p s w", w=W),
                        in_=src_b,
                    ).then_inc(sem_in[j], 16)

            @block.vector
            def _(vector):
                for k in range(NBLK):
                    j, r = k % NBUF, k // NBUF
                    vector.wait_ge(sem_in[j], 32 * (r + 1))
                    if k >= NBUF:
                        # output buffer free once block k-NBUF's stores ran
                        vector.wait_ge(sem_st[j], 32 * r)
                    t, o = tin[j], ob[j]
                    # even rows: ch1 @ even cols (seg1), ch2 @ odd cols (seg3)
                    vector.tensor_copy(
                        o[:, 0:W:2], t[:, W : 2 * W : 2]
                    ).then_inc(sem_cp, 1)
                    vector.tensor_copy(
                        o[:, 1:W:2], t[:, 3 * W + 1 : 4 * W : 2]
                    ).then_inc(sem_cp, 1)
                    # odd rows: ch0 @ even cols (seg0), ch1 @ odd cols (seg2)
                    vector.tensor_copy(
                        o[:, W : 2 * W : 2], t[:, 0:W:2]
                    ).then_inc(sem_cp, 1)
                    vector.tensor_copy(
                        o[:, W + 1 : 2 * W : 2], t[:, 2 * W + 1 : 3 * W : 2]
                    ).then_inc(sem_cp, 1)

            @block.scalar
            def _(scalar):
                # Each block's store is split in two 1 MiB halves (even rows
                # after copies 1-2, odd rows after copies 3-4) so the final
                # store chain starts as soon as possible -> shorter drain.
                for k in range(NBLK):
                    j, r = k % NBUF, k // NBUF
                    pr0 = k * P
                    scalar.wait_ge(sem_cp, 4 * k + 2)
                    if k >= NBUF:
                        scalar.wait_ge(sem_st[j], 32 * r)
                    scalar.dma_start(
                        out=out_r[pr0 : pr0 + P, 0, :],
                        in_=ob[j][:, 0:W],
                    ).then_inc(sem_st[j], 16)
                    scalar.wait_ge(sem_cp, 4 * k + 4)
                    scalar.dma_start(
                        out=out_r[pr0 : pr0 + P, 1, :],
                        in_=ob[j][:, W : 2 * W],
                    ).then_inc(sem_st[j], 16)

    return nc


def _get_nc():
    if not _NC_CACHE:
        _NC_CACHE.append(build_nc())
    return _NC_CACHE[0]


def kernel(**inputs) -> np.ndarray:
    img = np.asarray(inputs["img"], dtype=np.float32)
    assert img.shape == (B, C, H, W), img.shape

    from concourse.bass_utils import run_bass_kernel_spmd

    nc = _get_nc()
    in_maps = [{"img": np.ascontiguousarray(img[b])} for b in range(B)]
    res = run_bass_kernel_spmd(nc, in_maps, core_ids=list(range(B)))
    return np.stack([res.results[i]["out"] for i in range(B)], axis=0)
